# revision 31
# baseline (speedup 1.0000x reference)
"""Embedding lookup kernel for Trainium2 (8 NeuronCores, SPMD data-parallel).

Problem: out[b, s, :] = table[ids[b, s], :]
  ids:   [32, 8192] int32 (values in [0, 256))
  table: [256, 256] float32
  out:   [32, 8192, 256] float32

Strategy (data-parallel over tokens, hinted by the problem):
  - 262144 tokens split into 8 contiguous shards of 32768 tokens (4 batch
    rows per core).
  - Per core: DMA-gather rows from the HBM-resident table into SBUF using
    the token ids as descriptors (SWDGE InstDMAGatherAnt), then stream the
    gathered rows back to the output region with large contiguous HWDGE
    DMAs. Double-buffered, 8 chunks of 4096 tokens.
  - Host-side prep (cheap numpy): ids shard is cast to int16 and laid out
    in the wrapped-by-16-partitions order the gather engine expects, with
    the token order transposed so each SBUF partition accumulates a
    contiguous run of output tokens (contiguous 32 KiB stores/partition).
"""

import sys

if "/opt/trn_rl_repo" not in sys.path:
    sys.path.insert(0, "/opt/trn_rl_repo")

import numpy as np

BATCH, SEQ, VOCAB, EMBED = 32, 8192, 256, 256
N_CORES = 8
TOKENS = BATCH * SEQ                 # 262144
TOK_PER_CORE = TOKENS // N_CORES     # 32768
P = 128                              # SBUF partitions
TPP = TOK_PER_CORE // P              # tokens per partition = 256
NCHUNK = 32
CHUNK_TOK = TOK_PER_CORE // NCHUNK   # 1024 (SWDGE ring: <= scratch/16 descs/gather)
CHUNK_COLS = CHUNK_TOK // P          # 8 tokens per partition per chunk
NBUF = 3
DMA_SCRATCH = 49152                  # 3072-descriptor SWDGE ring

_CACHE = {}


def _build_program(repeats: int = 1):
    import concourse.bass as bass
    import concourse.mybir as mybir
    from concourse import bacc

    nc = bacc.Bacc("TRN2", target_bir_lowering=False, debug=False,
                   num_devices=N_CORES, dynamic_dma_scratch_size=DMA_SCRATCH)

    idx_d = nc.dram_tensor("idx", [P, TOK_PER_CORE // 16], mybir.dt.int16,
                           kind="ExternalInput")
    table_d = nc.dram_tensor("table", [VOCAB, EMBED], mybir.dt.float32,
                             kind="ExternalInput")
    out_d = nc.dram_tensor("out", [P, TPP * EMBED], mybir.dt.float32,
                           kind="ExternalOutput")

    with nc.Block() as block:
        idx_sb = nc.alloc_sbuf_tensor("idx_sb", [P, TOK_PER_CORE // 16],
                                      mybir.dt.int16)
        gbufs = [
            nc.alloc_sbuf_tensor(f"gbuf{b}", [P, CHUNK_COLS * EMBED],
                                 mybir.dt.float32)
            for b in range(NBUF)
        ]
        isem = nc.alloc_semaphore("isem")
        gsems = [nc.alloc_semaphore(f"gsem{b}") for b in range(NBUF)]
        osems = [nc.alloc_semaphore(f"osem{b}") for b in range(NBUF)]

        total = NCHUNK * repeats

        @block.gpsimd
        def _(gpsimd):
            gpsimd.dma_start(idx_sb.ap(), idx_d.ap()).then_inc(isem, 16)
            gpsimd.wait_ge(isem, 16)
            for k in range(total):
                b, m = k % NBUF, k // NBUF
                kk = k % NCHUNK
                if k >= NBUF:
                    # buffer b is free once chunk k-NBUF's store finished
                    gpsimd.wait_ge(osems[b], 16 * m)
                gb = gbufs[b]
                out_view = gb.ap().rearrange("p (c e) -> p c e", e=EMBED)
                idx_slice = idx_sb.ap()[:, kk * (CHUNK_TOK // 16):(kk + 1) * (CHUNK_TOK // 16)]
                gpsimd.dma_gather(
                    out_view,
                    table_d.ap(),
                    idx_slice,
                    num_idxs=CHUNK_TOK,
                    num_idxs_reg=CHUNK_TOK,
                    elem_size=EMBED,
                ).then_inc(gsems[b], 16)

        @block.sync
        def _(sync):
            for k in range(total):
                b, m = k % NBUF, k // NBUF
                kk = k % NCHUNK
                sync.wait_ge(gsems[b], 16 * (m + 1))
                sync.dma_start(
                    out_d.ap()[:, kk * CHUNK_COLS * EMBED:(kk + 1) * CHUNK_COLS * EMBED],
                    gbufs[b].ap(),
                ).then_inc(osems[b], 16)
            for b in range(NBUF):
                sync.wait_ge(osems[b], 16 * ((total + NBUF - 1 - b) // NBUF))

    nc.compile()
    return nc


def _build_program_b(repeats: int = 1, hilo: bool = False, ps_blocks: int = 2,
                     bc_bufs: int = 3, ps_bufs: int = 5, oh_bufs: int = 6,
                     ob_bufs: int = 4, sg: int = 1, wdt: str = "f32r"):
    """Plan B: one-hot @ table matmul from an SBUF-resident table.

    Per 512-token group: PE broadcasts ids across partitions (K=1 f32r
    matmul into PSUM), DVE builds the transposed one-hot with per-partition
    iota compares (f32r), PE contracts one-hot @ table (f32r, optionally a
    second hi/lo residual pass for ~1e-8 accuracy), ACT copies PSUM->SBUF,
    HWDGE streams results out. Only HBM traffic: ids in (128 KiB) + out
    (32 MiB) per core.
    """
    import concourse.bass as bass
    import concourse.mybir as mybir
    import concourse.tile as tile
    from concourse import bacc

    f32, f32r = mybir.dt.float32, mybir.dt.float32r
    mdt = {"f32r": f32r, "bf16": mybir.dt.bfloat16}[wdt]
    GT = 512                      # tokens per group
    NG = TOK_PER_CORE // GT       # 64 groups
    BPG = GT // P                 # 4 blocks per group

    nc = bacc.Bacc("TRN2", target_bir_lowering=False, debug=False,
                   num_devices=N_CORES)

    idsf_d = nc.dram_tensor("idsf", [1, TOK_PER_CORE], f32r,
                            kind="ExternalInput")
    table_d = nc.dram_tensor("table", [VOCAB, EMBED], f32,
                             kind="ExternalInput")
    ones_d = nc.dram_tensor("ones", [1, P], f32r, kind="ExternalInput")
    iota_d = nc.dram_tensor("iota", [P, 2], f32, kind="ExternalInput")
    out_d = nc.dram_tensor("out", [P, TPP * EMBED], mybir.dt.float32,
                           kind="ExternalOutput")

    with tile.TileContext(nc) as tc:
        with (
            tc.tile_pool(name="const", bufs=1) as const,
            tc.tile_pool(name="ohp", bufs=oh_bufs) as ohp,
            tc.tile_pool(name="obp", bufs=ob_bufs) as obp,
            tc.tile_pool(name="bcp", bufs=bc_bufs, space="PSUM") as bcp,
            tc.tile_pool(name="psp", bufs=ps_bufs, space="PSUM") as psp,
        ):
            idsf = const.tile([1, TOK_PER_CORE], f32r)
            nc.sync.dma_start(idsf[:], idsf_d.ap())
            ones = const.tile([1, P], f32r)
            nc.sync.dma_start(ones[:], ones_d.ap())
            iota2 = const.tile([P, 2], f32)
            nc.sync.dma_start(iota2[:], iota_d.ap())
            tb_raw = const.tile([P, 2 * EMBED], f32)
            nc.sync.dma_start(tb_raw[:, 0:EMBED], table_d.ap()[0:P, :])
            nc.sync.dma_start(tb_raw[:, EMBED:2 * EMBED], table_d.ap()[P:VOCAB, :])
            tb_hi = const.tile([P, 2 * EMBED], mdt)
            nc.vector.tensor_copy(tb_hi[:], tb_raw[:])
            if hilo:
                tb_lo = const.tile([P, 2 * EMBED], mdt)
                # residual = raw - hi, rounded to the matmul dtype
                hi_back = const.tile([P, 2 * EMBED], f32)
                nc.vector.tensor_copy(hi_back[:], tb_hi[:])
                nc.vector.tensor_tensor(tb_lo[:], tb_raw[:], hi_back[:],
                                        mybir.AluOpType.subtract)

            import contextlib

            def one_pass():
                ob = None
                for g in range(NG):
                    bc = bcp.tile([P, GT], mybir.dt.float32, tag="bc")
                    nc.tensor.matmul(bc[:], ones[:],
                                     idsf[:, g * GT:(g + 1) * GT],
                                     start=True, stop=True)
                    oh = ohp.tile([P, 2 * GT], mdt, tag="oh")
                    nc.vector.tensor_scalar(oh[:, 0:GT], bc[:],
                                            iota2[:, 0:1], None,
                                            mybir.AluOpType.is_equal)
                    nc.vector.tensor_scalar(oh[:, GT:2 * GT], bc[:],
                                            iota2[:, 1:2], None,
                                            mybir.AluOpType.is_equal)
                    nonlocal_ofs = (g % sg) * BPG * EMBED
                    if g % sg == 0:
                        ob = obp.tile([P, sg * BPG * EMBED], mybir.dt.float32,
                                      tag="ob")
                    for hh in range(BPG // ps_blocks):
                        ps = psp.tile([P, ps_blocks * EMBED],
                                      mybir.dt.float32, tag="ps")
                        for jj in range(ps_blocks):
                            j = hh * ps_blocks + jj
                            s = j * P
                            mms = [(0, tb_hi), (1, tb_hi)]
                            if hilo:
                                mms += [(0, tb_lo), (1, tb_lo)]
                            for mi, (v, tbl) in enumerate(mms):
                                nc.tensor.matmul(
                                    ps[:, jj * EMBED:(jj + 1) * EMBED],
                                    oh[:, v * GT + s: v * GT + s + P],
                                    tbl[:, v * EMBED:(v + 1) * EMBED],
                                    start=(mi == 0), stop=(mi == len(mms) - 1),
                                )
                        nc.scalar.copy(
                            ob[:, nonlocal_ofs + hh * ps_blocks * EMBED:
                               nonlocal_ofs + (hh + 1) * ps_blocks * EMBED],
                            ps[:])
                    if g % sg == sg - 1:
                        g0 = g - (sg - 1)
                        nc.sync.dma_start(
                            out_d.ap()[:, g0 * BPG * EMBED:(g + 1) * BPG * EMBED],
                            ob[:])

            if repeats == 1:
                one_pass()
            else:
                with tc.For_i(0, repeats, 1):
                    one_pass()

    nc.compile()
    return nc


def _prep_idsf(shard_ids: np.ndarray) -> np.ndarray:
    """[32768] ids -> [1, 32768] f32 in interleaved feed order."""
    fed = shard_ids.reshape(P, TPP).T.reshape(1, -1)
    return np.ascontiguousarray(fed.astype(np.float32))


def _plan_b_consts():
    iota = np.stack([np.arange(P, dtype=np.float32),
                     np.arange(P, 2 * P, dtype=np.float32)], axis=1)
    return {
        "ones": np.ones((1, P), np.float32),
        "iota": np.ascontiguousarray(iota),
    }


def _prep_idx(shard_ids: np.ndarray) -> np.ndarray:
    """shard_ids: [32768] int -> [128, 2048] int16 in gather feed order.

    Feed order: fed[i] = shard[(i % 128) * TPP + i // 128] so the gather
    (which writes token i to partition i%128, column i//128) leaves each
    partition holding a contiguous run of TPP output tokens.
    Wrapped %16 across partitions, replicated into all 8 16-partition groups.
    """
    fed = shard_ids.reshape(P, TPP).T.reshape(-1)          # [32768]
    t16 = fed.reshape(TOK_PER_CORE // 16, 16).T            # [16, 2048]
    return np.ascontiguousarray(np.tile(t16, (8, 1)).astype(np.int16))


def kernel(inputs: np.ndarray, kernel: np.ndarray) -> np.ndarray:
    from concourse.bass_utils import run_bass_kernel_spmd

    ids = np.asarray(inputs, dtype=np.int32).reshape(-1)
    table = np.ascontiguousarray(np.asarray(kernel, dtype=np.float32))

    if "nc" not in _CACHE:
        _CACHE["nc"] = _build_program_b(1)
    nc = _CACHE["nc"]

    consts = _plan_b_consts()
    in_maps = []
    for c in range(N_CORES):
        shard = ids[c * TOK_PER_CORE:(c + 1) * TOK_PER_CORE]
        in_maps.append({"idsf": _prep_idsf(shard), "table": table, **consts})

    res = run_bass_kernel_spmd(nc, in_maps, core_ids=list(range(N_CORES)))
    _CACHE["last_results"] = res

    parts = []
    for c in range(N_CORES):
        o = res.results[c]["out"]                  # [128, 65536] f32
        parts.append(o.reshape(TOK_PER_CORE, EMBED))
    return np.concatenate(parts, axis=0).reshape(BATCH, SEQ, EMBED)
